# revision 3
# baseline (speedup 1.0000x reference)
"""Mixtral-style sparse MoE layer on 8 Trainium2 NeuronCores.

Strategy: expert-parallel. The router (hidden @ gate_w, top-2, softmax) is
tiny (~0.04% of total FLOPs) and runs on host as part of sharding: tokens
are gathered per selected expert ("all-to-all by expert" done host-side
since inputs are full/unsharded anyway). Core e runs expert e's gated-MLP
FFN over its (capacity-padded) token batch; the host scatter-adds the
routed outputs with their routing weights. This performs the *sparse* MoE
compute (top-2 of 8 experts => 4x fewer FLOPs than the dense reference).

Per-core Bass kernel, fully fused single pass:
  - xT [H, C] tokens resident in SBUF as [128, 8, C]
  - for each 512-wide F block: stream w1/v/w2 block once, compute
    g = w1_b^T x, u = v_b^T x (PSUM, contract H), h = silu(g)*u (ACT+DVE),
    then down-project h through w2_b and accumulate out in SBUF (DVE add).
  - matmuls in float32r (full PE rate; fp32 storage with reduced-precision
    multiply) or float32 (4x slower, bit-accurate) -- MM_DT below.
"""

import numpy as np

import concourse.bacc as bacc
import concourse.mybir as mybir
import concourse.tile as tile
from concourse.bass_utils import run_bass_kernel_spmd

E, TOPK, H, F = 8, 2, 1024, 3584
B, S = 2, 2048
N_CORES = 8

KH = H // 128          # 8 contraction subtiles for the up-projection
F_TILE = 512           # F block streamed per outer iteration
NFO = F // F_TILE      # 7
FS = F_TILE // 128     # 4
HO = H // 128          # 8 output row tiles
TOK = 384              # token tile (>=256 keeps float32r at full PE rate)

MM_DT = mybir.dt.float32r  # matmul operand dtype (fp32 bits, fast multiply)

_nc_cache: dict = {}


def build_ffn(C: int, reps: int = 1):
    """Bass program: gated-MLP FFN for one expert over C tokens.

    xT [H, C] -> outT [H, C] with outT = w2^T (silu(w1^T x) * (v^T x)).
    `reps` repeats the whole computation (for amortized HW timing).
    """
    assert C % TOK == 0
    NT = C // TOK
    nc = bacc.Bacc("TRN2", target_bir_lowering=False, debug=False)
    f32 = mybir.dt.float32

    xT = nc.dram_tensor("xT", [H, C], MM_DT, kind="ExternalInput")
    w1 = nc.dram_tensor("w1", [H, F], MM_DT, kind="ExternalInput")
    v = nc.dram_tensor("v", [H, F], MM_DT, kind="ExternalInput")
    w2 = nc.dram_tensor("w2", [F, H], MM_DT, kind="ExternalInput")
    outT = nc.dram_tensor("outT", [H, C], f32, kind="ExternalOutput")

    with tile.TileContext(nc) as tc:
        with (
            tc.tile_pool(name="xp", bufs=1) as xp,
            tc.tile_pool(name="wp", bufs=2) as wp,
            tc.tile_pool(name="hhp", bufs=2) as hhp,
            tc.tile_pool(name="sgp", bufs=3) as sgp,
            tc.tile_pool(name="oap", bufs=1) as oap,
            tc.tile_pool(name="ps", bufs=2, space="PSUM") as ps,
        ):
            for _ in range(reps):
                x_sb = xp.tile([128, KH, C], MM_DT, tag="x")
                nc.sync.dma_start(
                    out=x_sb, in_=xT.ap().rearrange("(kh p) c -> p kh c", p=128)
                )
                x_mm = x_sb
                out_acc = oap.tile([128, HO, C], f32, tag="oa")

                for fo in range(NFO):
                    fsl = slice(fo * F_TILE, (fo + 1) * F_TILE)
                    w1_f = wp.tile([128, KH, F_TILE], MM_DT, tag="w1")
                    v_f = wp.tile([128, KH, F_TILE], MM_DT, tag="v")
                    w2_f = wp.tile([128, FS, H], MM_DT, tag="w2")
                    nc.sync.dma_start(
                        out=w1_f,
                        in_=w1.ap()[:, fsl].rearrange("(kh p) f -> p kh f", p=128),
                    )
                    nc.sync.dma_start(
                        out=v_f,
                        in_=v.ap()[:, fsl].rearrange("(kh p) f -> p kh f", p=128),
                    )
                    nc.sync.dma_start(
                        out=w2_f,
                        in_=w2.ap()[fsl, :].rearrange("(fs p) h -> p fs h", p=128),
                    )
                    w1_mm, v_mm, w2_mm = w1_f, v_f, w2_f

                    for t in range(NT):
                        tsl = slice(t * TOK, (t + 1) * TOK)
                        hh = hhp.tile([128, FS, TOK], MM_DT, tag="hh")
                        for fs in range(FS):
                            pg = ps.tile([128, TOK], f32, tag="pg")
                            pu = ps.tile([128, TOK], f32, tag="pu")
                            for kh in range(KH):
                                nc.tensor.matmul(
                                    pg,
                                    w1_mm[:, kh, fs * 128 : (fs + 1) * 128],
                                    x_mm[:, kh, tsl],
                                    start=(kh == 0),
                                    stop=(kh == KH - 1),
                                )
                            for kh in range(KH):
                                nc.tensor.matmul(
                                    pu,
                                    v_mm[:, kh, fs * 128 : (fs + 1) * 128],
                                    x_mm[:, kh, tsl],
                                    start=(kh == 0),
                                    stop=(kh == KH - 1),
                                )
                            sg = sgp.tile([128, TOK], f32, tag="sg")
                            nc.scalar.activation(
                                out=sg, in_=pg, func=mybir.ActivationFunctionType.Silu
                            )
                            nc.vector.tensor_mul(hh[:, fs, :], sg, pu)
                        hh_mm = hh
                        for ho in range(HO):
                            po = ps.tile([128, TOK], f32, tag="po", bufs=4)
                            for fs in range(FS):
                                nc.tensor.matmul(
                                    po,
                                    w2_mm[:, fs, ho * 128 : (ho + 1) * 128],
                                    hh_mm[:, fs, :],
                                    start=(fs == 0),
                                    stop=(fs == FS - 1),
                                )
                            if fo == 0:
                                nc.scalar.copy(out=out_acc[:, ho, tsl], in_=po)
                            else:
                                nc.vector.tensor_add(
                                    out_acc[:, ho, tsl], out_acc[:, ho, tsl], po
                                )

                nc.sync.dma_start(
                    out=outT.ap().rearrange("(ho p) c -> p ho c", p=128), in_=out_acc
                )
    nc.finalize()
    return nc


def get_ffn(C: int, reps: int = 1):
    key = (C, reps, str(MM_DT))
    if key not in _nc_cache:
        _nc_cache[key] = build_ffn(C, reps)
    return _nc_cache[key]


def route_host(hidden_states, gate_w, pressure_bias, temperature_field):
    """Router on host. Values in fp32 (matching reference numerics); the
    top-2 *ranking* in fp64 so borderline near-ties resolve to the true
    mathematical order rather than platform-dependent fp32 rounding."""
    hs = np.asarray(hidden_states, np.float32).reshape(-1, H)
    gw = np.asarray(gate_w, np.float32)
    pb = np.asarray(pressure_bias, np.float32)
    tf = np.asarray(temperature_field, np.float32)
    temp = np.clip(tf, 0.1, 10.0)

    logits64 = (hs.astype(np.float64) @ gw.astype(np.float64) + pb.astype(np.float64)) / temp.astype(np.float64)
    sel = np.argsort(-logits64, axis=-1, kind="stable")[:, :TOPK].astype(np.int32)

    logits32 = ((hs @ gw) + pb) / temp
    top_vals = np.take_along_axis(logits32, sel, axis=-1)
    mx = top_vals.max(axis=-1, keepdims=True)
    ex = np.exp(top_vals - mx)
    rw = (ex / ex.sum(axis=-1, keepdims=True)).astype(np.float32)
    return sel, rw


def kernel(hidden_states, gate_w, w1, v, w2, pressure_bias, temperature_field):
    hs = np.asarray(hidden_states, np.float32).reshape(-1, H)
    w1 = np.asarray(w1, np.float32)
    v = np.asarray(v, np.float32)
    w2 = np.asarray(w2, np.float32)
    n_tok = hs.shape[0]

    sel, rw = route_host(hidden_states, gate_w, pressure_bias, temperature_field)

    # token ids routed to each expert
    idx = [np.where((sel == e).any(axis=1))[0] for e in range(E)]
    counts = np.array([len(i) for i in idx])
    C = max(TOK, int(np.ceil(counts.max() / TOK)) * TOK)

    in_maps = []
    for e in range(E):
        xTe = np.zeros((H, C), np.float32)
        xTe[:, : counts[e]] = hs[idx[e]].T
        in_maps.append(
            {
                "xT": xTe,
                "w1": np.ascontiguousarray(w1[e]),
                "v": np.ascontiguousarray(v[e]),
                "w2": np.ascontiguousarray(w2[e]),
            }
        )

    nc = get_ffn(C)
    res = run_bass_kernel_spmd(nc, in_maps, core_ids=list(range(N_CORES)))

    out = np.zeros((n_tok, H), np.float32)
    for e in range(E):
        if counts[e] == 0:
            continue
        oe = res.results[e]["outT"][:, : counts[e]].T  # [n_e, H]
        we = np.where(sel[idx[e], 0] == e, rw[idx[e], 0], rw[idx[e], 1])
        out[idx[e]] += we[:, None] * oe

    return (
        out.reshape(B, S, H),
        rw.reshape(B, S, TOPK),
        sel.reshape(B, S, TOPK),
    )
